# revision 21
# baseline (speedup 1.0000x reference)
"""Trainium2 Bass kernel for AttnBlock3D (GroupNorm + per-frame spatial attention).

x: [1, 512, 16, 32, 32] fp32. 16 frames sharded 2-per-core across 8 NeuronCores.
GroupNorm stats span all frames -> tiny (4KB) AllReduce of per-channel partial
sums. All big matmuls run in fp8e4m3 with DoubleRow perf mode (K=256 per pass,
~1.5x bf16 column rate); accumulation in fp32 PSUM; residual path fp32.

Factored attention per frame (n=1024 tokens, c=512), using host-precomputed
M = 128*s*Wq^T@Wk (fp8), W2T = 16*(Wo@Wv)^T (fp8), bo2 = bo + Wo@bv:
  h    = GN(x)                     (per-channel scale/bias a,b; fp8)
  Z    = M^T h                     (one projection replaces Q and K; fp8 of
                                    128*Z_true via direct PSUM copy)
  S'   = h^T Z = 128*S_true        (DoubleRow, PSUM fp32)
  A'   = exp(S'/128 - 2)           (ACT scale+bias fused; fp8, max ~58 < 240)
  V2T  = h^T W2T                   (= 16*(W2 h)^T, j on partitions -> no
                                    transposes anywhere; fp8)
  l'   = 16 * sum_j A'             (ones=16 matmul; free partition bcast)
  Y    = V2T^T A' = 16 * W2 h A'   (DoubleRow over j)
  y    = Y/l' + bo2 + x            (the 16s and the exp shift cancel in Y/l')
The bk term is a per-i factor that cancels in the softmax over j; bq enters
via col_j = s*bq^T Wk h_j (wcol = 128*s*Wk^T@bq, fp8) -- skipped when bq == 0.
"""
import sys
sys.path.insert(0, '/opt/trn_rl_repo')
import numpy as np
import ml_dtypes

import concourse.bass as bass
import concourse.mybir as mybir
import concourse.tile as tile
from concourse import bacc
from concourse.bass_utils import run_bass_kernel_spmd

N_CORES = 8
C = 512            # channels
T = 16             # frames
SP = 1024          # tokens per frame (32*32)
FPC = T // N_CORES # frames per core = 2
G = 32             # groups
GS = C // G        # channels per group = 16
EPS = 1e-6
SCALE = float(C) ** -0.5
CB = C // 128      # channel blocks = 4
NP = CB // 2       # DoubleRow channel-slab pairs = 2
JC = SP // 128     # token chunks = 8
JP = JC // 2       # token-chunk pairs = 4
NH = SP // 512     # 512-wide halves = 2
NTOT = GS * T * SP # elements per group for GN stats

M_SCALE = 128.0    # host scale on M (and wcol)
W2_SCALE = 16.0    # host scale on W2T
ONES_VAL = W2_SCALE  # l' = 16*sum(A'); r = 1/l' makes Y*r = y_att exactly
S0 = 2.0           # exp shift: A' = exp(S - 2), max ~e^4 = 55 << 240

f32 = mybir.dt.float32
bf16 = mybir.dt.bfloat16
fp8 = mybir.dt.float8e4
AX = mybir.AxisListType
ALU = mybir.AluOpType
ACT = mybir.ActivationFunctionType
DR = mybir.MatmulPerfMode.DoubleRow


def build_program(repeats=1, sim_mode=False, with_col=False, fake_cc=False,
                  no_bo2=False):
    nc = bacc.Bacc("TRN2", target_bir_lowering=False, debug=False,
                   num_devices=(1 if sim_mode else N_CORES))
    xs = nc.dram_tensor("xs", [C, FPC, SP], f32, kind="ExternalInput").ap()
    m_in = nc.dram_tensor("m_in", [NP, 128, 2, C], fp8, kind="ExternalInput").ap()
    w2t = nc.dram_tensor("w2t", [NP, 128, 2, C], fp8, kind="ExternalInput").ap()
    wcol = nc.dram_tensor("wcol", [NP, 128, 2], fp8, kind="ExternalInput").ap()
    bo2 = nc.dram_tensor("bo2", [128, CB], f32, kind="ExternalInput").ap()
    gam = nc.dram_tensor("gam", [128, CB], f32, kind="ExternalInput").ap()
    bet = nc.dram_tensor("bet", [128, CB], f32, kind="ExternalInput").ap()
    onesd = nc.dram_tensor("onesd", [128, 2, 128], fp8, kind="ExternalInput").ap()
    gseld = nc.dram_tensor("gseld", [128, 8], f32, kind="ExternalInput").ap()
    sel2d = nc.dram_tensor("sel2d", [128, 128], f32, kind="ExternalInput").ap()
    ys = nc.dram_tensor("ys", [C, FPC, SP], f32, kind="ExternalOutput").ap()

    with tile.TileContext(nc) as tc:
        with (
            tc.tile_pool(name="const", bufs=1) as cpool,
            tc.tile_pool(name="xp", bufs=2 * CB) as xpool,
            tc.tile_pool(name="hp", bufs=2 * NP) as hpool,
            tc.tile_pool(name="zp", bufs=2 * NP) as zpool,
            tc.tile_pool(name="atp", bufs=2 * JP) as atpool,
            tc.tile_pool(name="vp", bufs=2 * JP) as vpool,
            tc.tile_pool(name="rp", bufs=3) as rpool,
            tc.tile_pool(name="yp", bufs=6) as ypool,
            tc.tile_pool(name="sp", bufs=3) as spool,
            tc.tile_pool(name="jp", bufs=3) as jpool,
            tc.tile_pool(name="pmm", bufs=3, space="PSUM") as pmm,
            tc.tile_pool(name="pl", bufs=1, space="PSUM") as plp,
            tc.tile_pool(name="dr", bufs=6, space="DRAM") as dpool,
        ):
            for rep_ in range(repeats):
                # ---- load x: one [128, FPC*SP] tile per channel block ----
                x_sb = []
                x_eng = [nc.sync, nc.sync, nc.sync, nc.sync]
                for b in range(CB):
                    t = xpool.tile([128, FPC, SP], f32, tag="x",
                                   name=f"x_{b}_{rep_}")
                    x_eng[b].dma_start(t[:], xs[b * 128:(b + 1) * 128, :, :])
                    x_sb.append(t)

                # ---- GN stats: per-channel sum / sumsq over both frames ----
                arin = spool.tile([128, CB, 2], f32, tag="arin",
                                  name=f"arin_{rep_}")
                for b in range(CB):
                    # keep the PE HAM warm through the stats preamble
                    pw = plp.tile([128, SP], f32, tag="pl",
                                  name=f"pwarm_{b}_{rep_}")
                    nc.tensor.matmul(pw[:, 0:2], x_sb[b][:, 0, 0:128],
                                     x_sb[b][:, 0, 0:2], start=True, stop=True)
                    nc.vector.reduce_sum(out=arin[:, b, 0:1], in_=x_sb[b][:],
                                         axis=AX.XY)
                    jt = jpool.tile([128, FPC, SP], bf16, tag="junk",
                                    name=f"junk_{b}_{rep_}")
                    nc.scalar.activation(out=jt[:], in_=x_sb[b][:],
                                         func=ACT.Square,
                                         accum_out=arin[:, b, 1:2])

                # ---- AllReduce of per-channel (sum, sumsq): 4KB ----
                cc_in = dpool.tile([128, CB * 2], f32, tag="ccin",
                                   name=f"ccin_{rep_}")
                cc_out = dpool.tile([128 * N_CORES, CB * 2], f32, tag="ccout",
                                    name=f"ccout_{rep_}")
                pw2 = plp.tile([128, SP], f32, tag="pl",
                               name=f"pwarm2_{rep_}")
                nc.tensor.matmul(pw2[0:2, 0:2], arin[:, 0, :], arin[:, 0, :],
                                 start=True, stop=True)
                nc.sync.dma_start(cc_in[:], arin[:].rearrange("p b s -> p (b s)"))
                if sim_mode or fake_cc:
                    for r in range(N_CORES):
                        nc.gpsimd.dma_start(cc_out[r * 128:(r + 1) * 128, :],
                                            cc_in[:])
                else:
                    # AllGather (~4.6us floor) + local reduce beats the
                    # ~10us AllReduce floor for this 4KB payload.
                    nc.gpsimd.collective_compute(
                        "AllGather", ALU.bypass,
                        replica_groups=[list(range(N_CORES))],
                        ins=[cc_in.opt()], outs=[cc_out.opt()],
                    )
                # gather per-rank slices [p, rank, (b s)] and sum over ranks
                csr = spool.tile([128, N_CORES, CB * 2], f32, tag="csr",
                                 name=f"csr_{rep_}")
                nc.sync.dma_start(csr[:], bass.AP(
                    tensor=cc_out.tensor, offset=cc_out.offset,
                    ap=[[CB * 2, 128], [128 * CB * 2, N_CORES], [1, CB * 2]]))
                cssum = spool.tile([128, CB, 2], f32, tag="cssum",
                                   name=f"cssum_{rep_}")
                nc.vector.reduce_sum(
                    out=cssum[:].rearrange("p b s -> p (b s)"),
                    in_=csr[:].rearrange("p r c -> p c r"), axis=AX.X)
                if rep_ == 0:
                # ---- constants ----
                    m_sb = [cpool.tile([128, 2, C], fp8, tag=f"m{p}",
                                       name=f"m_sb_{p}")
                            for p in range(NP)]
                    for p in range(NP):
                        nc.sync.dma_start(m_sb[p][:], m_in[p])
                    w2_sb = [cpool.tile([128, 2, C], fp8, tag=f"w2{p}",
                                        name=f"w2_sb_{p}")
                             for p in range(NP)]
                    for p in range(NP):
                        nc.sync.dma_start(w2_sb[p][:], w2t[p])
                    wcol_sb = cpool.tile([128, NP, 2], fp8, tag="wcol")
                    nc.sync.dma_start(
                        wcol_sb[:], wcol.rearrange("np p s -> p np s"))
                    bo2_sb = cpool.tile([128, CB], f32, tag="bo2")
                    nc.sync.dma_start(bo2_sb[:], bo2)
                    gam_sb = cpool.tile([128, CB], f32, tag="gam")
                    nc.sync.dma_start(gam_sb[:], gam)
                    bet_sb = cpool.tile([128, CB], f32, tag="bet")
                    nc.sync.dma_start(bet_sb[:], bet)
                    ones_sb = cpool.tile([128, 2, 128], fp8, tag="ones")
                    nc.sync.dma_start(ones_sb[:], onesd)
                    gsel_sb = cpool.tile([128, 8], f32, tag="gsel")
                    nc.sync.dma_start(gsel_sb[:], gseld)
                    sel2_sb = cpool.tile([128, 128], f32, tag="sel2")
                    nc.sync.dma_start(sel2_sb[:], sel2d)
                    ms0 = cpool.tile([128, 1], f32, tag="ms0")
                    nc.vector.memset(ms0[:], -S0)


                # load reduced per-channel sums back (1:1), group-sum via
                # 0/1 matrix G on the PE (fp32), stats math on 8 partitions,
                # then broadcast group->channel via G^T (fp32 matmul).
                pg = plp.tile([128, SP], f32, tag="pl", name=f"pg_{rep_}")
                nc.tensor.matmul(pg[0:8, 0:8], gsel_sb[:], cssum[:].rearrange(
                    "p b s -> p (b s)"), start=True, stop=True)
                # mean = s1/N ; rstd = 1/sqrt(s2/N - mean^2 + eps)  on [8, CB]
                pgv = pg[0:8, 0:8].rearrange("g (b s) -> g b s", s=2)
                mr8 = spool.tile([128, CB, 2], f32, tag="mr8", name=f"mr8_{rep_}")
                nc.scalar.mul(mr8[0:8, :, 0], pgv[:, :, 0], 1.0 / NTOT)
                ex2 = spool.tile([128, CB], f32, tag="ex2", name=f"ex2_{rep_}")
                nc.scalar.mul(ex2[0:8, :], pgv[:, :, 1], 1.0 / NTOT)
                msq = spool.tile([128, CB], f32, tag="msq", name=f"msq_{rep_}")
                nc.vector.tensor_mul(msq[0:8, :], mr8[0:8, :, 0], mr8[0:8, :, 0])
                var = spool.tile([128, CB], f32, tag="var", name=f"var_{rep_}")
                nc.vector.tensor_tensor(var[0:8, :], ex2[0:8, :], msq[0:8, :],
                                        ALU.subtract)
                sd = spool.tile([128, CB], f32, tag="sd", name=f"sd_{rep_}")
                eps_t = spool.tile([128, 1], f32, tag="eps",
                                   name=f"eps_{rep_}")
                nc.vector.memset(eps_t[0:8, :], EPS)
                nc.scalar.activation(out=sd[0:8, :], in_=var[0:8, :],
                                     func=ACT.Sqrt, bias=eps_t[0:8, :],
                                     scale=1.0)
                nc.vector.reciprocal(out=mr8[0:8, :, 1], in_=sd[0:8, :])
                pmr = plp.tile([128, SP], f32, tag="pl", name=f"pmr_{rep_}")
                nc.tensor.matmul(pmr[:, 0:8], sel2_sb[:],
                                 mr8[:].rearrange("g b s -> g (b s)"),
                                 start=True, stop=True)
                mr_ch = spool.tile([128, CB, 2], f32, tag="mrch",
                                   name=f"mrch_{rep_}")
                nc.scalar.copy(out=mr_ch[:],
                               in_=pmr[:, 0:8].rearrange("p (b s) -> p b s", s=2))
                # a = rstd*gamma ; b = beta - mean*a
                a_ch = spool.tile([128, CB], f32, tag="ach", name=f"ach_{rep_}")
                nc.vector.tensor_tensor(a_ch[:], mr_ch[:, :, 1], gam_sb[:],
                                        ALU.mult)
                bb_t = spool.tile([128, CB], f32, tag="bbt", name=f"bbt_{rep_}")
                nc.vector.tensor_tensor(bb_t[:], mr_ch[:, :, 0], a_ch[:],
                                        ALU.mult)
                b_ch = spool.tile([128, CB], f32, tag="bch", name=f"bch_{rep_}")
                nc.vector.tensor_tensor(b_ch[:], bet_sb[:], bb_t[:],
                                        ALU.subtract)

                # ---- GN apply for both frames: h = a*x + b (fp8, paired
                # c-slabs for DoubleRow) ----
                h2 = []
                for p in range(NP):
                    ht = hpool.tile([128, 2, FPC, SP], fp8, tag="h",
                                    name=f"h_{p}_{rep_}")
                    for s in range(2):
                        b = 2 * p + s
                        nc.vector.tensor_scalar(
                            out=ht[:, s], in0=x_sb[b][:],
                            scalar1=a_ch[:, b:b + 1], scalar2=b_ch[:, b:b + 1],
                            op0=ALU.mult, op1=ALU.add)
                    h2.append(ht)

                for f in range(FPC):
                    # Z = M^T h : PSUM holds 128*Z_true; copy -> fp8 pairs
                    z2 = [zpool.tile([128, 2, SP], fp8, tag="z",
                                     name=f"z_{f}_{p}_{rep_}")
                          for p in range(NP)]
                    for m in range(CB):
                        ps = pmm.tile([128, SP], f32, tag="mm",
                                      name=f"ps_z_{f}_{m}_{rep_}")
                        for hh in range(NH):
                            for p in range(NP):
                                nc.tensor.matmul(
                                    ps[:, hh * 512:(hh + 1) * 512],
                                    m_sb[p][:, :, m * 128:(m + 1) * 128],
                                    h2[p][:, :, f, hh * 512:(hh + 1) * 512],
                                    start=(p == 0), stop=(p == NP - 1),
                                    perf_mode=DR)
                        nc.vector.tensor_copy(out=z2[m // 2][:, m % 2, :],
                                              in_=ps[:])

                    # V2T = h^T W2T : [j-block 128, c2 512] -> fp8 pairs;
                    # fills the PE while DVE drains the Z copies.
                    v2 = [vpool.tile([128, 2, C], fp8, tag="v2",
                                     name=f"v2_{f}_{jj}_{rep_}")
                          for jj in range(JP)]
                    for jj in range(JP):
                        ps = pmm.tile([128, SP], f32, tag="mm",
                                      name=f"ps_v_{f}_{jj}_{rep_}")
                        for s in range(2):
                            j = 2 * jj + s
                            for p in range(NP):
                                nc.tensor.matmul(
                                    ps[:, s * 512:(s + 1) * 512],
                                    h2[p][:, :, f, j * 128:(j + 1) * 128],
                                    w2_sb[p][:, :, :],
                                    start=(p == 0), stop=(p == NP - 1),
                                    perf_mode=DR)
                        nc.vector.tensor_copy(
                            out=v2[jj][:].rearrange("p s c -> p (s c)"),
                            in_=ps[:])

                    # col_j = wcol^T h (only when bq != 0)
                    if with_col:
                        pcol = plp.tile([128, SP], f32, tag="pl",
                                        name=f"pcol_{f}_{rep_}")
                        for j in range(JC):
                            for b in range(CB):
                                nc.tensor.matmul(
                                    pcol[:, j:j + 1],
                                    h2[b // 2][:, b % 2, f,
                                               j * 128:(j + 1) * 128],
                                    wcol_sb[:, b // 2, b % 2:b % 2 + 1],
                                    start=(b == 0), stop=(b == CB - 1))
                        col_sb = rpool.tile([128, 8], f32, tag="col",
                                            name=f"col_{f}_{rep_}")
                        nc.scalar.activation(out=col_sb[:], in_=pcol[:, 0:8],
                                             func=ACT.Copy, bias=-S0,
                                             scale=1.0 / M_SCALE)

                    # S' per j-chunk (PSUM = 128*S); A' = exp(S - 2) fp8
                    at2 = [atpool.tile([128, 2, SP], fp8, tag="at",
                                       name=f"at_{f}_{jj}_{rep_}")
                           for jj in range(JP)]
                    for j in range(JC):
                        ps = pmm.tile([128, SP], f32, tag="mm",
                                      name=f"ps_s_{f}_{j}_{rep_}")
                        for hh in range(NH):
                            for p in range(NP):
                                nc.tensor.matmul(
                                    ps[:, hh * 512:(hh + 1) * 512],
                                    h2[p][:, :, f, j * 128:(j + 1) * 128],
                                    z2[p][:, :, hh * 512:(hh + 1) * 512],
                                    start=(p == 0), stop=(p == NP - 1),
                                    perf_mode=DR)
                        if with_col:
                            nc.scalar.activation(out=at2[j // 2][:, j % 2, :],
                                                 in_=ps[:], func=ACT.Exp,
                                                 scale=1.0 / M_SCALE,
                                                 bias=col_sb[:, j:j + 1])
                        else:
                            nc.scalar.activation(out=at2[j // 2][:, j % 2, :],
                                                 in_=ps[:], func=ACT.Exp,
                                                 scale=1.0 / M_SCALE,
                                                 bias=ms0[:])

                    # l'[i] broadcast to 128 partitions via ones(=16) matmul
                    pls = plp.tile([128, SP], f32, tag="pl",
                                   name=f"pl_{f}_{rep_}")
                    for hh in range(NH):
                        for jj in range(JP):
                            nc.tensor.matmul(
                                pls[:, hh * 512:(hh + 1) * 512], ones_sb[:],
                                at2[jj][:, :, hh * 512:(hh + 1) * 512],
                                start=(jj == 0), stop=(jj == JP - 1),
                                perf_mode=DR)
                    r_sb = rpool.tile([128, SP], f32, tag="r",
                                      name=f"r_{f}_{rep_}")
                    nc.vector.reciprocal(out=r_sb[:], in_=pls[:])

                    # Y[c2, i] = sum_j V2T[j, c2] A'[j, i]  (= 1024 * y_att)
                    for c2 in range(CB):
                        ps = pmm.tile([128, SP], f32, tag="mm",
                                      name=f"ps_y_{f}_{c2}_{rep_}")
                        for hh in range(NH):
                            for jj in range(JP):
                                nc.tensor.matmul(
                                    ps[:, hh * 512:(hh + 1) * 512],
                                    v2[jj][:, :, c2 * 128:(c2 + 1) * 128],
                                    at2[jj][:, :, hh * 512:(hh + 1) * 512],
                                    start=(jj == 0), stop=(jj == JP - 1),
                                    perf_mode=DR)
                        # y = Y*r + bo2 + x ; store (scales cancel in Y*r)
                        yt = ypool.tile([128, SP], f32, tag="y",
                                        name=f"y_{f}_{c2}_{rep_}")
                        tmp = jpool.tile([128, SP], f32, tag="ytmp",
                                         name=f"ytmp_{f}_{c2}_{rep_}")
                        nc.vector.tensor_tensor(tmp[:], ps[:], r_sb[:],
                                                ALU.mult)
                        nc.vector.scalar_tensor_tensor(
                            out=yt[:], in0=tmp[:],
                            scalar=bo2_sb[:, c2:c2 + 1],
                            in1=x_sb[c2][:, f, :],
                            op0=ALU.add, op1=ALU.add)
                        nc.sync.dma_start(ys[c2 * 128:(c2 + 1) * 128, f, :],
                                          yt[:])
    return nc


def _gsel():
    g = np.zeros((128, 8), np.float32)
    for p in range(128):
        g[p, p // GS] = 1.0
    return g


def _sel2():
    g = np.zeros((128, 128), np.float32)
    for p in range(128):
        g[p // GS, p] = 1.0
    return g


def _pair_lhsT(w, scale):
    """[C, C] (cin, cout) -> [NP, 128, 2, C] fp8 DoubleRow lhsT layout."""
    w = (scale * np.asarray(w, np.float32)).reshape(NP, 2, 128, C)
    return np.ascontiguousarray(w.transpose(0, 2, 1, 3)).astype(
        ml_dtypes.float8_e4m3)


def _host_inputs(x, gn_gamma, gn_beta, wq, bq, wk, bk, wv, bv, wo, bo):
    def pb(v, dt=np.float32):  # [C] -> [128, CB]
        return np.ascontiguousarray(
            np.asarray(v, np.float32).reshape(CB, 128).T).astype(dt)

    wq = np.asarray(wq, np.float32)
    wk = np.asarray(wk, np.float32)
    wv = np.asarray(wv, np.float32)
    wo = np.asarray(wo, np.float32)
    m_host = _pair_lhsT(SCALE * (wq.T @ wk), M_SCALE)       # [cin, cout]
    w2t_host = _pair_lhsT((wo @ wv).T, W2_SCALE)            # [cin, c2]
    wcol_host = (M_SCALE * SCALE * (wk.T @ np.asarray(bq, np.float32))
                 ).reshape(NP, 2, 128).transpose(0, 2, 1)
    wcol_host = np.ascontiguousarray(wcol_host).astype(ml_dtypes.float8_e4m3)
    bo2_host = pb(np.asarray(bo, np.float32) + wo @ np.asarray(bv, np.float32))

    shared = {
        "m_in": m_host,
        "w2t": w2t_host,
        "wcol": wcol_host,
        "bo2": bo2_host,
        "gam": pb(gn_gamma),
        "bet": pb(gn_beta),
        "onesd": np.full((128, 2, 128), ONES_VAL, ml_dtypes.float8_e4m3),
        "gseld": _gsel(),
        "sel2d": _sel2(),
    }
    xf = np.asarray(x, np.float32).reshape(C, T, SP)
    in_maps = []
    for i in range(N_CORES):
        m = dict(shared)
        m["xs"] = np.ascontiguousarray(xf[:, i * FPC:(i + 1) * FPC, :])
        in_maps.append(m)
    return in_maps


def run(inputs, repeats=1, nc=None):
    in_maps = _host_inputs(**inputs)
    if nc is None:
        with_col = bool(np.any(np.asarray(inputs["bq"], np.float32) != 0.0))
        bo2v = (np.asarray(inputs["bo"], np.float32)
                + np.asarray(inputs["wo"], np.float32)
                @ np.asarray(inputs["bv"], np.float32))
        nc = build_program(repeats, with_col=with_col,
                           no_bo2=not bool(np.any(bo2v != 0.0)))
        nc.compile()
    res = run_bass_kernel_spmd(nc, in_maps, core_ids=list(range(N_CORES)))
    out = np.empty((C, T, SP), np.float32)
    for i in range(N_CORES):
        out[:, i * FPC:(i + 1) * FPC, :] = res.results[i]["ys"]
    return out.reshape(1, C, T, 32, 32), res


def kernel(**inputs):
    out, _ = run(inputs)
    return out
